# revision 7
# baseline (speedup 1.0000x reference)
"""GCN layer (linear + BatchNorm1d(node) + copy_src/sum message passing + relu)
as a Trainium2 Bass kernel, data-parallel over the batch dim on 8 NeuronCores.

Math (reference):
    x = h @ W.T + b                      # (B, 3, 128)
    mean/var over (batch, feat) per node # training-mode BN stats
    xn = (x - mean) * rsqrt(var + eps) * gamma + beta
    out = relu(A @ xn per batch),  A[v,u] = #edges u->v

Device strategy (two passes over h, tiny all-reduce between):
  pass 1: accumulate per-node Gram matrices C_u = h_u^T h_u and column sums
          S_u = sum_b h_u via PE matmuls on natural-layout tiles.
          Stats follow from host-precomputed W-contractions:
            sum x    = S_u . wsum + B*sum(b)
            sum x^2  = <C_u, W^T W> + 2 S_u . (W^T b) + B*sum(b^2)
          All-reduce of 9 partial scalars across the 8 cores.
  pass 2: out[b] = relu(sum_u m3[v,u] * (h_u W^T) + bias2), with
          m3 = A*diag(s) folded into 3 "big weight" blocks m3[v,u]*W^T and
          bias2[v,f] = P_v*b[f] + q_v folded in via a K=1 ones matmul.
          h tiles are PE-transposed on the fly so the contraction dim (f_in)
          lands on partitions; output comes out in natural layout.
"""

import threading

import numpy as np

B_TOTAL = 262144
NN = 3
F = 128
FW = NN * F  # 384
N_CORES = 8
B_LOC = B_TOTAL // N_CORES  # 32768
CHUNK = 512  # batches per chunk per core
BN_EPS = 1e-5

_runner = None
_runner_lock = threading.Lock()


def _build_bass(b_loc, chunk):
    import concourse.bass as bass
    import concourse.tile as tile
    from concourse import bacc, mybir
    from concourse.masks import make_identity

    f32 = mybir.dt.float32
    f32r = mybir.dt.float32r
    X = mybir.AxisListType.X
    nj = chunk // 128
    nchunk = b_loc // chunk

    nc = bacc.Bacc("TRN2", target_bir_lowering=False, debug=False,
                   num_devices=N_CORES)

    def ein(name, shape):
        return nc.dram_tensor(name, shape, f32, kind="ExternalInput").ap()

    h_d = ein("h0", [b_loc, FW])
    wt_d = ein("wt", [F, F])        # W^T (wt[k, f] = W[f, k])
    g_d = ein("gmat", [F, F])       # G = W^T @ W
    wsum_d = ein("wsum", [F, 1])    # sum_f W[f, :]
    bwv_d = ein("bwv", [F, 1])      # W^T @ b
    bvec_d = ein("bvec", [1, F])    # b
    afl_d = ein("afl", [1, 9])      # A[v,u] flattened v-major
    gam_d = ein("gam", [1, NN])
    bet_d = ein("bet", [1, NN])
    # [B*sum(b), B*sum(b^2), 1/(B*F), eps]
    cst_d = ein("cst", [1, 4])
    out_d = nc.dram_tensor("out0", [b_loc, FW], f32, kind="ExternalOutput").ap()

    with tile.TileContext(nc) as tc:
        with tc.tile_pool(name="singles", bufs=1) as singles:
            def load_single(src, shape, name):
                t = singles.tile(shape, f32, name=name, tag=name)
                nc.sync.dma_start(out=t, in_=src)
                return t

            wt_sb = load_single(wt_d, [F, F], "wt_sb")
            g_sb = load_single(g_d, [F, F], "g_sb")
            wsum_sb = load_single(wsum_d, [F, 1], "wsum_sb")
            bwv_sb = load_single(bwv_d, [F, 1], "bwv_sb")
            bvec_sb = load_single(bvec_d, [1, F], "bvec_sb")
            afl_sb = load_single(afl_d, [1, 9], "afl_sb")
            gam_sb = load_single(gam_d, [1, NN], "gam_sb")
            bet_sb = load_single(bet_d, [1, NN], "bet_sb")
            cst_sb = load_single(cst_d, [1, 4], "cst_sb")

            ident = singles.tile([128, 128], f32)
            make_identity(nc, ident)
            ones_col = singles.tile([128, 1], f32)
            nc.vector.memset(ones_col, 1.0)
            ones_rowf = singles.tile([1, 128], f32)
            nc.vector.memset(ones_rowf, 1.0)
            ones_row = singles.tile([1, 128], f32r)
            nc.vector.tensor_copy(out=ones_row, in_=ones_rowf)
            onesrep = None

            # ---------------- pass 1: Gram/sum accumulation ----------------
            red = singles.tile([128, 9], f32)   # cols: [q_u | sxw_u | sb_u] grouped by type
            arout = singles.tile([1, 9], f32)
            with tc.tile_pool(name="p1", bufs=3) as p1pool, \
                 tc.tile_pool(name="p1ps", bufs=1, space="PSUM") as p1ps:
                psc = [p1ps.tile([128, FW + 2], f32, tag=f"psc{u}", name=f"psc{u}")
                       for u in range(NN)]
                onesrep = singles.tile([128, nj, 2], f32, name="onesrep")
                nc.vector.memset(onesrep, 1.0)
                for c in range(nchunk):
                    ht = p1pool.tile([128, nj, FW + 2], f32r, tag="ht")
                    nc.sync.dma_start(
                        out=ht[:, :, 0:FW],
                        in_=h_d[c * chunk:(c + 1) * chunk, :].rearrange(
                            "(p j) f -> p j f", j=nj).bitcast(f32r),
                    )
                    nc.vector.tensor_copy(out=ht[:, :, FW:FW + 2], in_=onesrep)
                    for j in range(nj):
                        mov = ht[:, j, :]
                        for u in range(NN):
                            nc.tensor.matmul(
                                psc[u],
                                lhsT=ht[:, j, u * F:(u + 1) * F],
                                rhs=mov,
                                start=(c == 0 and j == 0),
                                stop=(c == nchunk - 1 and j == nj - 1),
                                skip_group_check=True,
                            )

                # local reductions: q_u = <C_uu, G>, sxw_u = S_u.wsum, sb_u = S_u.bW
                tmp = singles.tile([128, F], f32)
                for u in range(NN):
                    nc.vector.tensor_mul(tmp, psc[u][:, u * F:(u + 1) * F], g_sb)
                    nc.vector.reduce_sum(out=red[:, u:u + 1], in_=tmp, axis=X)
                    nc.vector.tensor_mul(red[:, 3 + u:4 + u],
                                         psc[u][:, FW:FW + 1], wsum_sb)
                    nc.vector.tensor_mul(red[:, 6 + u:7 + u],
                                         psc[u][:, FW:FW + 1], bwv_sb)

                with tc.tile_pool(name="eps", bufs=1, space="PSUM") as epsum:
                    ps_red = epsum.tile([1, 9], f32)
                    nc.tensor.matmul(ps_red, lhsT=ones_col, rhs=red,
                                     start=True, stop=True)
                    arin = singles.tile([1, 9], f32)
                    nc.vector.tensor_copy(out=arin, in_=ps_red)

                    with tc.tile_pool(name="dram", bufs=1, space="DRAM") as drp:
                        bounce_in = drp.tile([1, 9], f32)
                        bounce_out = drp.tile([1, 9], f32)
                        nc.sync.dma_start(out=bounce_in, in_=arin)
                        nc.gpsimd.collective_compute(
                            "AllReduce",
                            mybir.AluOpType.add,
                            replica_groups=[list(range(N_CORES))],
                            ins=[bounce_in[:].opt()],
                            outs=[bounce_out[:].opt()],
                        )
                        nc.sync.dma_start(out=arout, in_=bounce_out)

            # ---------------- stats -> folded weights ----------------
            _small_n = [0]

            def small(shape=(1, NN)):
                _small_n[0] += 1
                return singles.tile(list(shape), f32,
                                    name=f"stat{_small_n[0]}")

            mean = small()
            # mean = (sxw + B*sum(b)) / (B*F)
            nc.vector.tensor_scalar(out=mean, in0=arout[:, 3:6],
                                    scalar1=cst_sb[:, 0:1], scalar2=cst_sb[:, 2:3],
                                    op0=mybir.AluOpType.add,
                                    op1=mybir.AluOpType.mult)
            # e2 = (q + 2*sb + B*sum(b^2)) / (B*F)
            t0 = small()
            nc.vector.tensor_add(t0, arout[:, 0:3], arout[:, 6:9])
            nc.vector.tensor_add(t0, t0, arout[:, 6:9])
            e2 = small()
            nc.vector.tensor_scalar(out=e2, in0=t0,
                                    scalar1=cst_sb[:, 1:2], scalar2=cst_sb[:, 2:3],
                                    op0=mybir.AluOpType.add,
                                    op1=mybir.AluOpType.mult)
            var = small()
            nc.vector.tensor_mul(var, mean, mean)
            nc.vector.tensor_sub(var, e2, var)
            sd = small()
            nc.scalar.activation(out=sd, in_=var,
                                 func=mybir.ActivationFunctionType.Sqrt,
                                 bias=cst_sb[:, 3:4], scale=1.0)
            rs = small()
            nc.vector.reciprocal(rs, sd)
            s_sb = small()
            nc.vector.tensor_mul(s_sb, gam_sb, rs)

            def rep3(t):
                # [1,3] -> [1,3,3] view repeating along the new middle dim
                return bass.AP(tensor=t.tensor, offset=t.offset,
                               ap=[t.ap[0], [0, NN], t.ap[-1]])

            afl3 = bass.AP(tensor=afl_sb.tensor, offset=afl_sb.offset,
                           ap=[afl_sb.ap[0], [NN, NN], [1, NN]])
            m3 = singles.tile([1, NN, NN], f32)  # m3[v,u] = A[v,u]*s_u
            nc.vector.tensor_mul(m3, afl3, rep3(s_sb))
            pv = small()
            nc.vector.reduce_sum(out=pv, in_=m3, axis=X)
            tb = small()
            nc.vector.tensor_mul(tb, s_sb, mean)
            nc.vector.tensor_sub(tb, bet_sb, tb)
            qt = singles.tile([1, NN, NN], f32)
            nc.vector.tensor_mul(qt, afl3, rep3(tb))
            qv = small()
            nc.vector.reduce_sum(out=qv, in_=qt, axis=X)

            bias2 = singles.tile([1, FW], f32r)
            for v in range(NN):
                nc.vector.tensor_scalar(out=bias2[:, v * F:(v + 1) * F],
                                        in0=bvec_sb,
                                        scalar1=pv[:, v:v + 1],
                                        scalar2=qv[:, v:v + 1],
                                        op0=mybir.AluOpType.mult,
                                        op1=mybir.AluOpType.add)

            m3b = singles.tile([128, 9], f32)
            bwc = [singles.tile([128, FW], f32r, tag=f"bwc{u}", name=f"bwc{u}")
                   for u in range(NN)]
            with tc.tile_pool(name="bps", bufs=1, space="PSUM") as bps:
                ps_b = bps.tile([128, 9], f32)
                nc.tensor.matmul(ps_b, lhsT=ones_rowf,
                                 rhs=m3.rearrange("p a b -> p (a b)"),
                                 start=True, stop=True)
                nc.vector.tensor_copy(out=m3b, in_=ps_b)
                for u in range(NN):
                    for v in range(NN):
                        nc.vector.tensor_scalar_mul(
                            out=bwc[u][:, v * F:(v + 1) * F], in0=wt_sb,
                            scalar1=m3b[:, v * NN + u:v * NN + u + 1])

            # ---------------- pass 2: out = relu(sum_u hT_u^T @ bwc_u + bias2) ----
            with tc.tile_pool(name="p2", bufs=3) as p2pool, \
                 tc.tile_pool(name="p2t", bufs=3) as p2t, \
                 tc.tile_pool(name="p2ps", bufs=3, space="PSUM") as p2ps, \
                 tc.tile_pool(name="p2pst", bufs=4, space="PSUM") as p2pst:
                for c in range(nchunk):
                    ht2 = p2pool.tile([128, nj, FW], f32, tag="ht2")
                    nc.sync.dma_start(
                        out=ht2,
                        in_=h_d[c * chunk:(c + 1) * chunk, :].rearrange(
                            "(p j) f -> p j f", j=nj),
                    )
                    osb = p2pool.tile([128, nj, FW], f32, tag="osb")
                    for j in range(nj):
                        hT = p2t.tile([128, NN, 128], f32r, tag="hT")
                        for u in range(NN):
                            pst = p2pst.tile([128, 128], f32, tag="pst")
                            nc.tensor.transpose(
                                pst, ht2[:, j, u * F:(u + 1) * F], ident)
                            nc.vector.tensor_copy(out=hT[:, u, :], in_=pst)
                        pso = p2ps.tile([128, FW], f32, tag="pso")
                        nc.tensor.matmul(pso, lhsT=ones_row,
                                         rhs=bias2,
                                         start=True, stop=False,
                                         skip_group_check=True)
                        for u in range(NN):
                            nc.tensor.matmul(pso,
                                             lhsT=hT[:, u, :],
                                             rhs=bwc[u],
                                             start=False, stop=(u == NN - 1),
                                             skip_group_check=True)
                        nc.scalar.activation(
                            out=osb[:, j, :], in_=pso,
                            func=mybir.ActivationFunctionType.Relu)
                    nc.sync.dma_start(
                        out=out_d[c * chunk:(c + 1) * chunk, :].rearrange(
                            "(p j) f -> p j f", j=nj),
                        in_=osb)

    nc.finalize()
    return nc


class _Runner:
    """Caches the compiled 8-core PJRT executable across kernel() calls."""

    def __init__(self, b_loc=B_LOC, chunk=CHUNK):
        import jax
        from jax.sharding import Mesh, PartitionSpec
        from jax.experimental.shard_map import shard_map
        from concourse import bass2jax, mybir

        self.b_loc = b_loc
        nc = _build_bass(b_loc, chunk)
        bass2jax.install_neuronx_cc_hook()

        partition_name = (nc.partition_id_tensor.name
                          if nc.partition_id_tensor else None)
        in_names, out_names, out_avals, zero_outs = [], [], [], []
        for alloc in nc.m.functions[0].allocations:
            if not isinstance(alloc, mybir.MemoryLocationSet):
                continue
            name = alloc.memorylocations[0].name
            if alloc.kind == "ExternalInput":
                if name != partition_name:
                    in_names.append(name)
            elif alloc.kind == "ExternalOutput":
                shape = tuple(alloc.tensor_shape)
                dtype = mybir.dt.np(alloc.dtype)
                out_names.append(name)
                out_avals.append(jax.core.ShapedArray(shape, dtype))
                zero_outs.append(np.zeros(shape, dtype))
        self.in_names = list(in_names)
        self.out_names = out_names
        self.out_avals = out_avals
        self.zero_outs = zero_outs
        n_params = len(in_names)
        all_in_names = in_names + out_names
        if partition_name is not None:
            all_in_names.append(partition_name)

        def _body(*args):
            operands = list(args)
            if partition_name is not None:
                operands.append(bass2jax.partition_id_tensor())
            outs = bass2jax._bass_exec_p.bind(
                *operands,
                out_avals=tuple(out_avals),
                in_names=tuple(all_in_names),
                out_names=tuple(out_names),
                lowering_input_output_aliases=(),
                sim_require_finite=False,
                sim_require_nnan=False,
                nc=nc,
            )
            return tuple(outs)

        devices = jax.devices()[:N_CORES]
        assert len(devices) == N_CORES
        self.mesh = Mesh(np.asarray(devices), ("core",))
        n_all = n_params + len(out_names)
        self.fn = jax.jit(
            shard_map(_body, mesh=self.mesh,
                      in_specs=(PartitionSpec("core"),) * n_all,
                      out_specs=(PartitionSpec("core"),) * len(out_names),
                      check_rep=False),
            keep_unused=True,
        )
        self.jax = jax

    def concat_inputs(self, in_maps):
        concat = [
            np.concatenate([np.asarray(m[name]) for m in in_maps], axis=0)
            for name in self.in_names
        ]
        concat += [
            np.zeros((N_CORES * z.shape[0], *z.shape[1:]), z.dtype)
            for z in self.zero_outs
        ]
        return concat

    def run(self, in_maps):
        out_arrs = self.fn(*self.concat_inputs(in_maps))
        return [
            {name: np.asarray(out_arrs[i]).reshape(
                N_CORES, *self.out_avals[i].shape)[c]
             for i, name in enumerate(self.out_names)}
            for c in range(N_CORES)
        ]


def _host_prep(h, W, b, gamma, beta, src, dst, b_total):
    """Host-side tiny precomputations (O(F^2), no O(B) work)."""
    W = np.asarray(W, np.float32)
    b = np.asarray(b, np.float32)
    A = np.zeros((NN, NN), np.float32)
    np.add.at(A, (np.asarray(dst).astype(np.int64),
                  np.asarray(src).astype(np.int64)), 1.0)
    smalls = {
        "wt": np.ascontiguousarray(W.T),
        "gmat": np.ascontiguousarray(W.T @ W),
        "wsum": np.ascontiguousarray(W.sum(axis=0)[:, None]),
        "bwv": np.ascontiguousarray((W * b[:, None]).sum(axis=0)[:, None]),
        "bvec": np.ascontiguousarray(b[None, :]),
        "afl": np.ascontiguousarray(A.reshape(1, 9)),
        "gam": np.ascontiguousarray(np.asarray(gamma, np.float32)[None, :]),
        "bet": np.ascontiguousarray(np.asarray(beta, np.float32)[None, :]),
        "cst": np.array([[b_total * float(b.sum()),
                          b_total * float((b * b).sum()),
                          1.0 / (b_total * F),
                          BN_EPS]], np.float32),
    }
    return smalls


def _get_runner():
    global _runner
    with _runner_lock:
        if _runner is None:
            _runner = _Runner()
        return _runner


def kernel(h, W, b, gamma, beta, src, dst):
    h = np.asarray(h, np.float32)
    assert h.shape == (B_TOTAL, NN, F), h.shape
    runner = _get_runner()
    smalls = _host_prep(h, W, b, gamma, beta, src, dst, B_TOTAL)
    hf = np.ascontiguousarray(h.reshape(B_TOTAL, FW))
    in_maps = []
    for c in range(N_CORES):
        m = dict(smalls)
        m["h0"] = hf[c * B_LOC:(c + 1) * B_LOC]
        in_maps.append(m)
    outs = runner.run(in_maps)
    full = np.concatenate([outs[c]["out0"] for c in range(N_CORES)], axis=0)
    return full.reshape(B_TOTAL, NN, F)


# revision 14
# speedup vs baseline: 1.6046x; 1.6046x over previous
"""GCN layer (linear + BatchNorm1d(node) + copy_src/sum message passing + relu)
as a Trainium2 Bass kernel, data-parallel over the batch dim on 8 NeuronCores.

Math (reference):
    x = h @ W.T + b                      # (B, 3, 128)
    mean/var over (batch, feat) per node # training-mode BN stats
    xn = (x - mean) * rsqrt(var + eps) * gamma + beta
    out = relu(A @ xn per batch),  A[v,u] = #edges u->v

Device strategy (two passes over h, tiny all-reduce between):
  pass 1: accumulate per-node Gram matrices C_u = h_u^T h_u and column sums
          S_u = sum_b h_u via PE matmuls on natural-layout tiles.
          Stats follow from host-precomputed W-contractions:
            sum x    = S_u . wsum + B*sum(b)
            sum x^2  = <C_u, W^T W> + 2 S_u . (W^T b) + B*sum(b^2)
          All-reduce of 9 partial scalars across the 8 cores.
  pass 2: out[b] = relu(sum_u m3[v,u] * (h_u W^T) + bias2), with
          m3 = A*diag(s) folded into 3 "big weight" blocks m3[v,u]*W^T and
          bias2[v,f] = P_v*b[f] + q_v folded in via a K=1 ones matmul.
          h tiles are PE-transposed on the fly so the contraction dim (f_in)
          lands on partitions; output comes out in natural layout.
"""

import threading

import numpy as np

B_TOTAL = 262144
NN = 3
F = 128
FW = NN * F  # 384
N_CORES = 8
B_LOC = B_TOTAL // N_CORES  # 32768
CHUNK = 512  # batches per chunk per core
BN_EPS = 1e-5

_runner = None
_runner_lock = threading.Lock()


def _build_bass(b_loc, chunk, trace_sim=False):
    import concourse.bass as bass
    import concourse.tile as tile
    from concourse import bacc, mybir
    from concourse.masks import make_identity

    f32 = mybir.dt.float32
    f32r = mybir.dt.float32r
    X = mybir.AxisListType.X
    nj = chunk // 128
    nchunk = b_loc // chunk

    nc = bacc.Bacc("TRN2", target_bir_lowering=False, debug=False,
                   num_devices=N_CORES)

    def ein(name, shape):
        return nc.dram_tensor(name, shape, f32, kind="ExternalInput").ap()

    h_d = ein("h0", [b_loc, FW])
    wt_d = ein("wt", [F, F])        # W^T (wt[k, f] = W[f, k])
    g_d = ein("gmat", [F, F])       # G = W^T @ W
    wsum_d = ein("wsum", [F, 1])    # sum_f W[f, :]
    bwv_d = ein("bwv", [F, 1])      # W^T @ b
    bvec_d = ein("bvec", [1, F])    # b
    afl_d = ein("afl", [1, 9])      # A[v,u] flattened v-major
    gam_d = ein("gam", [1, NN])
    bet_d = ein("bet", [1, NN])
    # [B*sum(b), B*sum(b^2), 1/(B*F), eps]
    cst_d = ein("cst", [1, 4])
    out_d = nc.dram_tensor("out0", [b_loc, FW], f32, kind="ExternalOutput").ap()

    with tile.TileContext(nc, trace_sim=trace_sim) as tc:
        with tc.tile_pool(name="singles", bufs=1) as singles:
            def load_single(src, shape, name):
                t = singles.tile(shape, f32, name=name, tag=name)
                nc.sync.dma_start(out=t, in_=src)
                return t

            wt_sb = load_single(wt_d, [F, F], "wt_sb")
            g_sb = load_single(g_d, [F, F], "g_sb")
            wsum_sb = load_single(wsum_d, [F, 1], "wsum_sb")
            bwv_sb = load_single(bwv_d, [F, 1], "bwv_sb")
            bvec_sb = load_single(bvec_d, [1, F], "bvec_sb")
            afl_sb = load_single(afl_d, [1, 9], "afl_sb")
            gam_sb = load_single(gam_d, [1, NN], "gam_sb")
            bet_sb = load_single(bet_d, [1, NN], "bet_sb")
            cst_sb = load_single(cst_d, [1, 4], "cst_sb")

            ident = singles.tile([128, 128], f32)
            make_identity(nc, ident)
            ones_col = singles.tile([128, 1], f32)
            nc.vector.memset(ones_col, 1.0)
            ones_rowf = singles.tile([1, 128], f32)
            nc.vector.memset(ones_rowf, 1.0)
            ones_row = singles.tile([1, 128], f32r)
            nc.vector.tensor_copy(out=ones_row, in_=ones_rowf)
            onesrep = None

            # ---------------- pass 1: Gram/sum accumulation ----------------
            # Last NKEEP chunks stay resident in SBUF and skip the pass-2
            # reload (saves NKEEP*chunk*FW*4 bytes of HBM re-read traffic).
            nkeep = min(16, nchunk)
            keep_start = nchunk - nkeep
            kept = {}
            red = singles.tile([128, 9], f32)   # cols: [q_u | sxw_u | sb_u] grouped by type
            arout = singles.tile([1, 9], f32)
            ctx_keep = tc.tile_pool(name="keep", bufs=nkeep)
            keep_pool = ctx_keep.__enter__()
            with tc.tile_pool(name="p1", bufs=3) as p1pool, \
                 tc.tile_pool(name="p1ps", bufs=1, space="PSUM") as p1ps:
                psc = [p1ps.tile([128, FW + 2], f32, tag=f"psc{u}", name=f"psc{u}")
                       for u in range(NN)]
                onesrep = singles.tile([128, nj, 2], f32, name="onesrep")
                nc.vector.memset(onesrep, 1.0)
                for c in range(nchunk):
                    if c >= keep_start:
                        ht = keep_pool.tile([128, nj, FW + 2], f32r, tag="htk",
                                            name=f"htk{c}")
                        kept[c] = ht
                    else:
                        ht = p1pool.tile([128, nj, FW + 2], f32r, tag="ht",
                                         name="ht")
                    nc.gpsimd.dma_start(
                        out=ht[:, :, 0:FW],
                        in_=h_d[c * chunk:(c + 1) * chunk, :].rearrange(
                            "(p j) f -> p j f", j=nj).bitcast(f32r),
                    )
                    nc.vector.tensor_copy(out=ht[:, :, FW:FW + 2], in_=onesrep)
                    for j in range(nj):
                        mov = ht[:, j, :]
                        for u in range(NN):
                            nc.tensor.matmul(
                                psc[u],
                                lhsT=ht[:, j, u * F:(u + 1) * F],
                                rhs=mov,
                                start=(c == 0 and j == 0),
                                stop=(c == nchunk - 1 and j == nj - 1),
                                skip_group_check=True,
                            )

                # local reductions: q_u = <C_uu, G>, sxw_u = S_u.wsum, sb_u = S_u.bW
                tmp = singles.tile([128, F], f32)
                for u in range(NN):
                    nc.vector.tensor_mul(tmp, psc[u][:, u * F:(u + 1) * F], g_sb)
                    nc.vector.reduce_sum(out=red[:, u:u + 1], in_=tmp, axis=X)
                    nc.vector.tensor_mul(red[:, 3 + u:4 + u],
                                         psc[u][:, FW:FW + 1], wsum_sb)
                    nc.vector.tensor_mul(red[:, 6 + u:7 + u],
                                         psc[u][:, FW:FW + 1], bwv_sb)

                with tc.tile_pool(name="eps", bufs=1, space="PSUM") as epsum:
                    ps_red = epsum.tile([1, 9], f32)
                    nc.tensor.matmul(ps_red, lhsT=ones_col, rhs=red,
                                     start=True, stop=True)
                    arin = singles.tile([1, 9], f32)
                    nc.vector.tensor_copy(out=arin, in_=ps_red)

                    with tc.tile_pool(name="dram", bufs=1, space="DRAM") as drp:
                        bounce_in = drp.tile([1, 9], f32)
                        bounce_out = drp.tile([1, 9], f32)
                        nc.sync.dma_start(out=bounce_in, in_=arin)
                        nc.gpsimd.collective_compute(
                            "AllReduce",
                            mybir.AluOpType.add,
                            replica_groups=[list(range(N_CORES))],
                            ins=[bounce_in[:].opt()],
                            outs=[bounce_out[:].opt()],
                        )
                        nc.sync.dma_start(out=arout, in_=bounce_out)

            # ---------------- stats -> folded weights ----------------
            _small_n = [0]

            def small(shape=(1, NN)):
                _small_n[0] += 1
                return singles.tile(list(shape), f32,
                                    name=f"stat{_small_n[0]}")

            mean = small()
            # mean = (sxw + B*sum(b)) / (B*F)
            nc.vector.tensor_scalar(out=mean, in0=arout[:, 3:6],
                                    scalar1=cst_sb[:, 0:1], scalar2=cst_sb[:, 2:3],
                                    op0=mybir.AluOpType.add,
                                    op1=mybir.AluOpType.mult)
            # e2 = (q + 2*sb + B*sum(b^2)) / (B*F)
            t0 = small()
            nc.vector.tensor_add(t0, arout[:, 0:3], arout[:, 6:9])
            nc.vector.tensor_add(t0, t0, arout[:, 6:9])
            e2 = small()
            nc.vector.tensor_scalar(out=e2, in0=t0,
                                    scalar1=cst_sb[:, 1:2], scalar2=cst_sb[:, 2:3],
                                    op0=mybir.AluOpType.add,
                                    op1=mybir.AluOpType.mult)
            var = small()
            nc.vector.tensor_mul(var, mean, mean)
            nc.vector.tensor_sub(var, e2, var)
            sd = small()
            nc.scalar.activation(out=sd, in_=var,
                                 func=mybir.ActivationFunctionType.Sqrt,
                                 bias=cst_sb[:, 3:4], scale=1.0)
            rs = small()
            nc.vector.reciprocal(rs, sd)
            s_sb = small()
            nc.vector.tensor_mul(s_sb, gam_sb, rs)

            def rep3(t):
                # [1,3] -> [1,3,3] view repeating along the new middle dim
                return bass.AP(tensor=t.tensor, offset=t.offset,
                               ap=[t.ap[0], [0, NN], t.ap[-1]])

            afl3 = bass.AP(tensor=afl_sb.tensor, offset=afl_sb.offset,
                           ap=[afl_sb.ap[0], [NN, NN], [1, NN]])
            m3 = singles.tile([1, NN, NN], f32)  # m3[v,u] = A[v,u]*s_u
            nc.vector.tensor_mul(m3, afl3, rep3(s_sb))
            pv = small()
            nc.vector.reduce_sum(out=pv, in_=m3, axis=X)
            tb = small()
            nc.vector.tensor_mul(tb, s_sb, mean)
            nc.vector.tensor_sub(tb, bet_sb, tb)
            qt = singles.tile([1, NN, NN], f32)
            nc.vector.tensor_mul(qt, afl3, rep3(tb))
            qv = small()
            nc.vector.reduce_sum(out=qv, in_=qt, axis=X)

            bias2 = singles.tile([1, FW], f32r)
            for v in range(NN):
                nc.vector.tensor_scalar(out=bias2[:, v * F:(v + 1) * F],
                                        in0=bvec_sb,
                                        scalar1=pv[:, v:v + 1],
                                        scalar2=qv[:, v:v + 1],
                                        op0=mybir.AluOpType.mult,
                                        op1=mybir.AluOpType.add)

            m3b = singles.tile([128, 9], f32)
            bwc = [singles.tile([128, FW], f32r, tag=f"bwc{u}", name=f"bwc{u}")
                   for u in range(NN)]
            with tc.tile_pool(name="bps", bufs=1, space="PSUM") as bps:
                ps_b = bps.tile([128, 9], f32)
                nc.tensor.matmul(ps_b, lhsT=ones_rowf,
                                 rhs=m3.rearrange("p a b -> p (a b)"),
                                 start=True, stop=True)
                nc.vector.tensor_copy(out=m3b, in_=ps_b)
                for u in range(NN):
                    for v in range(NN):
                        nc.vector.tensor_scalar_mul(
                            out=bwc[u][:, v * F:(v + 1) * F], in0=wt_sb,
                            scalar1=m3b[:, v * NN + u:v * NN + u + 1])

            # ---------------- pass 2: out = relu(sum_u hT_u^T @ bwc_u + bias2) ----
            identr = singles.tile([128, 128], f32r)
            nc.vector.tensor_copy(out=identr, in_=ident)
            with tc.tile_pool(name="p2", bufs=6) as p2pool, \
                 tc.tile_pool(name="p2t", bufs=3) as p2t, \
                 tc.tile_pool(name="p2ps", bufs=3, space="PSUM") as p2ps, \
                 tc.tile_pool(name="p2pst", bufs=4, space="PSUM") as p2pst:
                # retained chunks first: they need no DMA, so compute starts
                # while streamed chunks are still loading
                order = list(range(keep_start, nchunk)) + list(range(keep_start))
                for c in order:
                    if c in kept:
                        src = kept[c]
                    else:
                        src = p2pool.tile([128, nj, FW], f32r, tag="ht2",
                                          name="ht2")
                        nc.sync.dma_start(
                            out=src,
                            in_=h_d[c * chunk:(c + 1) * chunk, :].rearrange(
                                "(p j) f -> p j f", j=nj).bitcast(f32r),
                        )
                    osb = p2pool.tile([128, nj, FW], f32, tag="osb")
                    for j in range(nj):
                        hT = p2t.tile([128, NN, 128], f32r, tag="hT")
                        for u in range(NN):
                            pst = p2pst.tile([128, 128], f32r, tag="pst")
                            nc.tensor.transpose(
                                pst, src[:, j, u * F:(u + 1) * F], identr)
                            nc.vector.tensor_copy(out=hT[:, u, :], in_=pst)
                        pso = p2ps.tile([128, FW], f32, tag="pso")
                        nc.tensor.matmul(pso, lhsT=ones_row,
                                         rhs=bias2,
                                         start=True, stop=False,
                                         skip_group_check=True)
                        for u in range(NN):
                            nc.tensor.matmul(pso,
                                             lhsT=hT[:, u, :],
                                             rhs=bwc[u],
                                             start=False, stop=(u == NN - 1),
                                             skip_group_check=True)
                        nc.scalar.activation(
                            out=osb[:, j, :], in_=pso,
                            func=mybir.ActivationFunctionType.Relu)
                    nc.gpsimd.dma_start(
                        out=out_d[c * chunk:(c + 1) * chunk, :].rearrange(
                            "(p j) f -> p j f", j=nj),
                        in_=osb)
            ctx_keep.__exit__(None, None, None)

    nc.finalize()
    return nc


class _Runner:
    """Caches the compiled 8-core PJRT executable across kernel() calls."""

    def __init__(self, b_loc=B_LOC, chunk=CHUNK):
        import jax
        from jax.sharding import Mesh, PartitionSpec
        from jax.experimental.shard_map import shard_map
        from concourse import bass2jax, mybir

        self.b_loc = b_loc
        nc = _build_bass(b_loc, chunk)
        bass2jax.install_neuronx_cc_hook()

        partition_name = (nc.partition_id_tensor.name
                          if nc.partition_id_tensor else None)
        in_names, out_names, out_avals, zero_outs = [], [], [], []
        for alloc in nc.m.functions[0].allocations:
            if not isinstance(alloc, mybir.MemoryLocationSet):
                continue
            name = alloc.memorylocations[0].name
            if alloc.kind == "ExternalInput":
                if name != partition_name:
                    in_names.append(name)
            elif alloc.kind == "ExternalOutput":
                shape = tuple(alloc.tensor_shape)
                dtype = mybir.dt.np(alloc.dtype)
                out_names.append(name)
                out_avals.append(jax.core.ShapedArray(shape, dtype))
                zero_outs.append(np.zeros(shape, dtype))
        self.in_names = list(in_names)
        self.out_names = out_names
        self.out_avals = out_avals
        self.zero_outs = zero_outs
        n_params = len(in_names)
        all_in_names = in_names + out_names
        if partition_name is not None:
            all_in_names.append(partition_name)

        def _body(*args):
            operands = list(args)
            if partition_name is not None:
                operands.append(bass2jax.partition_id_tensor())
            outs = bass2jax._bass_exec_p.bind(
                *operands,
                out_avals=tuple(out_avals),
                in_names=tuple(all_in_names),
                out_names=tuple(out_names),
                lowering_input_output_aliases=(),
                sim_require_finite=False,
                sim_require_nnan=False,
                nc=nc,
            )
            return tuple(outs)

        devices = jax.devices()[:N_CORES]
        assert len(devices) == N_CORES
        self.mesh = Mesh(np.asarray(devices), ("core",))
        n_all = n_params + len(out_names)
        self.fn = jax.jit(
            shard_map(_body, mesh=self.mesh,
                      in_specs=(PartitionSpec("core"),) * n_all,
                      out_specs=(PartitionSpec("core"),) * len(out_names),
                      check_rep=False),
            keep_unused=True,
        )
        self.jax = jax

    def concat_inputs(self, in_maps):
        concat = [
            np.concatenate([np.asarray(m[name]) for m in in_maps], axis=0)
            for name in self.in_names
        ]
        concat += [
            np.zeros((N_CORES * z.shape[0], *z.shape[1:]), z.dtype)
            for z in self.zero_outs
        ]
        return concat

    def run(self, in_maps):
        out_arrs = self.fn(*self.concat_inputs(in_maps))
        return [
            {name: np.asarray(out_arrs[i]).reshape(
                N_CORES, *self.out_avals[i].shape)[c]
             for i, name in enumerate(self.out_names)}
            for c in range(N_CORES)
        ]


def _host_prep(h, W, b, gamma, beta, src, dst, b_total):
    """Host-side tiny precomputations (O(F^2), no O(B) work)."""
    W = np.asarray(W, np.float32)
    b = np.asarray(b, np.float32)
    A = np.zeros((NN, NN), np.float32)
    np.add.at(A, (np.asarray(dst).astype(np.int64),
                  np.asarray(src).astype(np.int64)), 1.0)
    smalls = {
        "wt": np.ascontiguousarray(W.T),
        "gmat": np.ascontiguousarray(W.T @ W),
        "wsum": np.ascontiguousarray(W.sum(axis=0)[:, None]),
        "bwv": np.ascontiguousarray((W * b[:, None]).sum(axis=0)[:, None]),
        "bvec": np.ascontiguousarray(b[None, :]),
        "afl": np.ascontiguousarray(A.reshape(1, 9)),
        "gam": np.ascontiguousarray(np.asarray(gamma, np.float32)[None, :]),
        "bet": np.ascontiguousarray(np.asarray(beta, np.float32)[None, :]),
        "cst": np.array([[b_total * float(b.sum()),
                          b_total * float((b * b).sum()),
                          1.0 / (b_total * F),
                          BN_EPS]], np.float32),
    }
    return smalls


def _get_runner():
    global _runner
    with _runner_lock:
        if _runner is None:
            _runner = _Runner()
        return _runner


def kernel(h, W, b, gamma, beta, src, dst):
    h = np.asarray(h, np.float32)
    assert h.shape == (B_TOTAL, NN, F), h.shape
    runner = _get_runner()
    smalls = _host_prep(h, W, b, gamma, beta, src, dst, B_TOTAL)
    hf = np.ascontiguousarray(h.reshape(B_TOTAL, FW))
    in_maps = []
    for c in range(N_CORES):
        m = dict(smalls)
        m["h0"] = hf[c * B_LOC:(c + 1) * B_LOC]
        in_maps.append(m)
    outs = runner.run(in_maps)
    full = np.concatenate([outs[c]["out0"] for c in range(N_CORES)], axis=0)
    return full.reshape(B_TOTAL, NN, F)


# revision 16
# speedup vs baseline: 133.3485x; 83.1037x over previous
"""GCN layer (linear + BatchNorm1d(node) + copy_src/sum message passing + relu)
as a Trainium2 Bass kernel, data-parallel over the batch dim on 8 NeuronCores.

Math (reference):
    x = h @ W.T + b                      # (B, 3, 128)
    mean/var over (batch, feat) per node # training-mode BN stats
    xn = (x - mean) * rsqrt(var + eps) * gamma + beta
    out = relu(A @ xn per batch),  A[v,u] = #edges u->v

Device strategy (two passes over h, tiny all-reduce between):
  pass 1: accumulate per-node Gram matrices C_u = h_u^T h_u and column sums
          S_u = sum_b h_u via PE matmuls on natural-layout tiles.
          Stats follow from host-precomputed W-contractions:
            sum x    = S_u . wsum + B*sum(b)
            sum x^2  = <C_u, W^T W> + 2 S_u . (W^T b) + B*sum(b^2)
          All-reduce of 9 partial scalars across the 8 cores.
  pass 2: out[b] = relu(sum_u m3[v,u] * (h_u W^T) + bias2), with
          m3 = A*diag(s) folded into 3 "big weight" blocks m3[v,u]*W^T and
          bias2[v,f] = P_v*b[f] + q_v folded in via a K=1 ones matmul.
          h tiles are PE-transposed on the fly so the contraction dim (f_in)
          lands on partitions; output comes out in natural layout.
"""

import threading

import numpy as np

B_TOTAL = 262144
NN = 3
F = 128
FW = NN * F  # 384
N_CORES = 8
B_LOC = B_TOTAL // N_CORES  # 32768
CHUNK = 512  # batches per chunk per core
BN_EPS = 1e-5

_runner = None
_runner_lock = threading.Lock()


def _build_bass(b_loc, chunk, trace_sim=False):
    import concourse.bass as bass
    import concourse.tile as tile
    from concourse import bacc, mybir
    from concourse.masks import make_identity

    f32 = mybir.dt.float32
    f32r = mybir.dt.float32r
    X = mybir.AxisListType.X
    nj = chunk // 128
    nchunk = b_loc // chunk

    nc = bacc.Bacc("TRN2", target_bir_lowering=False, debug=False,
                   num_devices=N_CORES)

    def ein(name, shape):
        return nc.dram_tensor(name, shape, f32, kind="ExternalInput").ap()

    h_d = ein("h0", [b_loc, FW])
    wt_d = ein("wt", [F, F])        # W^T (wt[k, f] = W[f, k])
    g_d = ein("gmat", [F, F])       # G = W^T @ W
    wsum_d = ein("wsum", [F, 1])    # sum_f W[f, :]
    bwv_d = ein("bwv", [F, 1])      # W^T @ b
    bvec_d = ein("bvec", [1, F])    # b
    afl_d = ein("afl", [1, 9])      # A[v,u] flattened v-major
    gam_d = ein("gam", [1, NN])
    bet_d = ein("bet", [1, NN])
    # [B*sum(b), B*sum(b^2), 1/(B*F), eps]
    cst_d = ein("cst", [1, 4])
    out_d = nc.dram_tensor("out0", [b_loc, FW], f32, kind="ExternalOutput").ap()

    with tile.TileContext(nc, trace_sim=trace_sim) as tc:
        with tc.tile_pool(name="singles", bufs=1) as singles:
            def load_single(src, shape, name):
                t = singles.tile(shape, f32, name=name, tag=name)
                nc.sync.dma_start(out=t, in_=src)
                return t

            wt_sb = load_single(wt_d, [F, F], "wt_sb")
            g_sb = load_single(g_d, [F, F], "g_sb")
            wsum_sb = load_single(wsum_d, [F, 1], "wsum_sb")
            bwv_sb = load_single(bwv_d, [F, 1], "bwv_sb")
            bvec_sb = load_single(bvec_d, [1, F], "bvec_sb")
            afl_sb = load_single(afl_d, [1, 9], "afl_sb")
            gam_sb = load_single(gam_d, [1, NN], "gam_sb")
            bet_sb = load_single(bet_d, [1, NN], "bet_sb")
            cst_sb = load_single(cst_d, [1, 4], "cst_sb")

            ident = singles.tile([128, 128], f32)
            make_identity(nc, ident)
            ones_col = singles.tile([128, 1], f32)
            nc.vector.memset(ones_col, 1.0)
            ones_rowf = singles.tile([1, 128], f32)
            nc.vector.memset(ones_rowf, 1.0)
            ones_row = singles.tile([1, 128], f32r)
            nc.vector.tensor_copy(out=ones_row, in_=ones_rowf)
            onesrep = None

            # ---------------- pass 1: Gram/sum accumulation ----------------
            # Last NKEEP chunks stay resident in SBUF and skip the pass-2
            # reload (saves NKEEP*chunk*FW*4 bytes of HBM re-read traffic).
            nkeep = min(16, nchunk)
            keep_start = nchunk - nkeep
            kept = {}
            red = singles.tile([128, 9], f32)   # cols: [q_u | sxw_u | sb_u] grouped by type
            arout = singles.tile([1, 9], f32)
            ctx_keep = tc.tile_pool(name="keep", bufs=nkeep)
            keep_pool = ctx_keep.__enter__()
            ctx_p1 = tc.tile_pool(name="p1", bufs=3)
            p1pool = ctx_p1.__enter__()
            with tc.tile_pool(name="p1ps", bufs=1, space="PSUM") as p1ps:
                psc = [p1ps.tile([128, FW + 2], f32, tag=f"psc{u}", name=f"psc{u}")
                       for u in range(NN)]
                onesrep = singles.tile([128, nj, 2], f32, name="onesrep")
                nc.vector.memset(onesrep, 1.0)
                for c in range(nchunk):
                    if c >= keep_start:
                        ht = keep_pool.tile([128, nj, FW + 2], f32r, tag="htk",
                                            name=f"htk{c}")
                        kept[c] = ht
                    else:
                        ht = p1pool.tile([128, nj, FW + 2], f32r, tag="ht",
                                         name="ht")
                        if c >= keep_start - 3:
                            kept[c] = ht
                    nc.gpsimd.dma_start(
                        out=ht[:, :, 0:FW],
                        in_=h_d[c * chunk:(c + 1) * chunk, :].rearrange(
                            "(p j) f -> p j f", j=nj).bitcast(f32r),
                    )
                    nc.vector.tensor_copy(out=ht[:, :, FW:FW + 2], in_=onesrep)
                    for j in range(nj):
                        mov = ht[:, j, :]
                        for u in range(NN):
                            nc.tensor.matmul(
                                psc[u],
                                lhsT=ht[:, j, u * F:(u + 1) * F],
                                rhs=mov,
                                start=(c == 0 and j == 0),
                                stop=(c == nchunk - 1 and j == nj - 1),
                                skip_group_check=True,
                            )

                # local reductions: q_u = <C_uu, G>, sxw_u = S_u.wsum, sb_u = S_u.bW
                tmp = singles.tile([128, F], f32)
                for u in range(NN):
                    nc.vector.tensor_mul(tmp, psc[u][:, u * F:(u + 1) * F], g_sb)
                    nc.vector.reduce_sum(out=red[:, u:u + 1], in_=tmp, axis=X)
                    nc.vector.tensor_mul(red[:, 3 + u:4 + u],
                                         psc[u][:, FW:FW + 1], wsum_sb)
                    nc.vector.tensor_mul(red[:, 6 + u:7 + u],
                                         psc[u][:, FW:FW + 1], bwv_sb)

                with tc.tile_pool(name="eps", bufs=1, space="PSUM") as epsum:
                    ps_red = epsum.tile([1, 9], f32)
                    nc.tensor.matmul(ps_red, lhsT=ones_col, rhs=red,
                                     start=True, stop=True)
                    arin = singles.tile([1, 9], f32)
                    nc.vector.tensor_copy(out=arin, in_=ps_red)

                    with tc.tile_pool(name="dram", bufs=1, space="DRAM") as drp:
                        bounce_in = drp.tile([1, 9], f32)
                        bounce_out = drp.tile([1, 9], f32)
                        nc.sync.dma_start(out=bounce_in, in_=arin)
                        nc.gpsimd.collective_compute(
                            "AllReduce",
                            mybir.AluOpType.add,
                            replica_groups=[list(range(N_CORES))],
                            ins=[bounce_in[:].opt()],
                            outs=[bounce_out[:].opt()],
                        )
                        nc.sync.dma_start(out=arout, in_=bounce_out)

            # ---------------- stats -> folded weights ----------------
            _small_n = [0]

            def small(shape=(1, NN)):
                _small_n[0] += 1
                return singles.tile(list(shape), f32,
                                    name=f"stat{_small_n[0]}")

            mean = small()
            # mean = (sxw + B*sum(b)) / (B*F)
            nc.vector.tensor_scalar(out=mean, in0=arout[:, 3:6],
                                    scalar1=cst_sb[:, 0:1], scalar2=cst_sb[:, 2:3],
                                    op0=mybir.AluOpType.add,
                                    op1=mybir.AluOpType.mult)
            # e2 = (q + 2*sb + B*sum(b^2)) / (B*F)
            t0 = small()
            nc.vector.tensor_add(t0, arout[:, 0:3], arout[:, 6:9])
            nc.vector.tensor_add(t0, t0, arout[:, 6:9])
            e2 = small()
            nc.vector.tensor_scalar(out=e2, in0=t0,
                                    scalar1=cst_sb[:, 1:2], scalar2=cst_sb[:, 2:3],
                                    op0=mybir.AluOpType.add,
                                    op1=mybir.AluOpType.mult)
            var = small()
            nc.vector.tensor_mul(var, mean, mean)
            nc.vector.tensor_sub(var, e2, var)
            sd = small()
            nc.scalar.activation(out=sd, in_=var,
                                 func=mybir.ActivationFunctionType.Sqrt,
                                 bias=cst_sb[:, 3:4], scale=1.0)
            rs = small()
            nc.vector.reciprocal(rs, sd)
            s_sb = small()
            nc.vector.tensor_mul(s_sb, gam_sb, rs)

            def rep3(t):
                # [1,3] -> [1,3,3] view repeating along the new middle dim
                return bass.AP(tensor=t.tensor, offset=t.offset,
                               ap=[t.ap[0], [0, NN], t.ap[-1]])

            afl3 = bass.AP(tensor=afl_sb.tensor, offset=afl_sb.offset,
                           ap=[afl_sb.ap[0], [NN, NN], [1, NN]])
            m3 = singles.tile([1, NN, NN], f32)  # m3[v,u] = A[v,u]*s_u
            nc.vector.tensor_mul(m3, afl3, rep3(s_sb))
            pv = small()
            nc.vector.reduce_sum(out=pv, in_=m3, axis=X)
            tb = small()
            nc.vector.tensor_mul(tb, s_sb, mean)
            nc.vector.tensor_sub(tb, bet_sb, tb)
            qt = singles.tile([1, NN, NN], f32)
            nc.vector.tensor_mul(qt, afl3, rep3(tb))
            qv = small()
            nc.vector.reduce_sum(out=qv, in_=qt, axis=X)

            bias2 = singles.tile([1, FW], f32r)
            for v in range(NN):
                nc.vector.tensor_scalar(out=bias2[:, v * F:(v + 1) * F],
                                        in0=bvec_sb,
                                        scalar1=pv[:, v:v + 1],
                                        scalar2=qv[:, v:v + 1],
                                        op0=mybir.AluOpType.mult,
                                        op1=mybir.AluOpType.add)

            m3b = singles.tile([128, 9], f32)
            bwc = [singles.tile([128, FW], f32r, tag=f"bwc{u}", name=f"bwc{u}")
                   for u in range(NN)]
            with tc.tile_pool(name="bps", bufs=1, space="PSUM") as bps:
                ps_b = bps.tile([128, 9], f32)
                nc.tensor.matmul(ps_b, lhsT=ones_rowf,
                                 rhs=m3.rearrange("p a b -> p (a b)"),
                                 start=True, stop=True)
                nc.vector.tensor_copy(out=m3b, in_=ps_b)
                for u in range(NN):
                    for v in range(NN):
                        nc.vector.tensor_scalar_mul(
                            out=bwc[u][:, v * F:(v + 1) * F], in0=wt_sb,
                            scalar1=m3b[:, v * NN + u:v * NN + u + 1])

            # ---------------- pass 2: out = relu(sum_u hT_u^T @ bwc_u + bias2) ----
            identr = singles.tile([128, 128], f32r)
            nc.vector.tensor_copy(out=identr, in_=ident)
            with tc.tile_pool(name="p2", bufs=6) as p2pool, \
                 tc.tile_pool(name="p2t", bufs=3) as p2t, \
                 tc.tile_pool(name="p2ps", bufs=3, space="PSUM") as p2ps, \
                 tc.tile_pool(name="p2pst", bufs=4, space="PSUM") as p2pst:
                # retained chunks first: they need no DMA, so compute starts
                # while streamed chunks are still loading
                order = list(range(keep_start, nchunk)) + list(range(keep_start))
                for c in order:
                    if c in kept:
                        src = kept[c]
                    else:
                        src = p2pool.tile([128, nj, FW], f32r, tag="ht2",
                                          name="ht2")
                        nc.sync.dma_start(
                            out=src,
                            in_=h_d[c * chunk:(c + 1) * chunk, :].rearrange(
                                "(p j) f -> p j f", j=nj).bitcast(f32r),
                        )
                    osb = p2pool.tile([128, nj, FW], f32, tag="osb")
                    for j in range(nj):
                        hT = p2t.tile([128, NN, 128], f32r, tag="hT")
                        for u in range(NN):
                            pst = p2pst.tile([128, 128], f32r, tag="pst")
                            nc.tensor.transpose(
                                pst, src[:, j, u * F:(u + 1) * F], identr)
                            nc.vector.tensor_copy(out=hT[:, u, :], in_=pst)
                        pso = p2ps.tile([128, FW], f32, tag="pso")
                        nc.tensor.matmul(pso, lhsT=ones_row,
                                         rhs=bias2,
                                         start=True, stop=False,
                                         skip_group_check=True)
                        for u in range(NN):
                            nc.tensor.matmul(pso,
                                             lhsT=hT[:, u, :],
                                             rhs=bwc[u],
                                             start=False, stop=(u == NN - 1),
                                             skip_group_check=True)
                        nc.scalar.activation(
                            out=osb[:, j, :], in_=pso,
                            func=mybir.ActivationFunctionType.Relu)
                    nc.gpsimd.dma_start(
                        out=out_d[c * chunk:(c + 1) * chunk, :].rearrange(
                            "(p j) f -> p j f", j=nj),
                        in_=osb)
            ctx_p1.__exit__(None, None, None)
            ctx_keep.__exit__(None, None, None)

    nc.finalize()
    return nc


class _Runner:
    """Caches the compiled 8-core PJRT executable across kernel() calls."""

    def __init__(self, b_loc=B_LOC, chunk=CHUNK):
        import jax
        from jax.sharding import Mesh, PartitionSpec
        from jax.experimental.shard_map import shard_map
        from concourse import bass2jax, mybir

        self.b_loc = b_loc
        nc = _build_bass(b_loc, chunk)
        bass2jax.install_neuronx_cc_hook()

        partition_name = (nc.partition_id_tensor.name
                          if nc.partition_id_tensor else None)
        in_names, out_names, out_avals, zero_outs = [], [], [], []
        for alloc in nc.m.functions[0].allocations:
            if not isinstance(alloc, mybir.MemoryLocationSet):
                continue
            name = alloc.memorylocations[0].name
            if alloc.kind == "ExternalInput":
                if name != partition_name:
                    in_names.append(name)
            elif alloc.kind == "ExternalOutput":
                shape = tuple(alloc.tensor_shape)
                dtype = mybir.dt.np(alloc.dtype)
                out_names.append(name)
                out_avals.append(jax.core.ShapedArray(shape, dtype))
                zero_outs.append(np.zeros(shape, dtype))
        self.in_names = list(in_names)
        self.out_names = out_names
        self.out_avals = out_avals
        self.zero_outs = zero_outs
        n_params = len(in_names)
        all_in_names = in_names + out_names
        if partition_name is not None:
            all_in_names.append(partition_name)

        def _body(*args):
            operands = list(args)
            if partition_name is not None:
                operands.append(bass2jax.partition_id_tensor())
            outs = bass2jax._bass_exec_p.bind(
                *operands,
                out_avals=tuple(out_avals),
                in_names=tuple(all_in_names),
                out_names=tuple(out_names),
                lowering_input_output_aliases=(),
                sim_require_finite=False,
                sim_require_nnan=False,
                nc=nc,
            )
            return tuple(outs)

        devices = jax.devices()[:N_CORES]
        assert len(devices) == N_CORES
        self.mesh = Mesh(np.asarray(devices), ("core",))
        n_all = n_params + len(out_names)
        self.fn = jax.jit(
            shard_map(_body, mesh=self.mesh,
                      in_specs=(PartitionSpec("core"),) * n_all,
                      out_specs=(PartitionSpec("core"),) * len(out_names),
                      check_rep=False),
            keep_unused=True,
        )
        self.jax = jax

    def concat_inputs(self, in_maps):
        concat = [
            np.concatenate([np.asarray(m[name]) for m in in_maps], axis=0)
            for name in self.in_names
        ]
        concat += [
            np.zeros((N_CORES * z.shape[0], *z.shape[1:]), z.dtype)
            for z in self.zero_outs
        ]
        return concat

    def run(self, in_maps):
        out_arrs = self.fn(*self.concat_inputs(in_maps))
        return [
            {name: np.asarray(out_arrs[i]).reshape(
                N_CORES, *self.out_avals[i].shape)[c]
             for i, name in enumerate(self.out_names)}
            for c in range(N_CORES)
        ]


def _host_prep(h, W, b, gamma, beta, src, dst, b_total):
    """Host-side tiny precomputations (O(F^2), no O(B) work)."""
    W = np.asarray(W, np.float32)
    b = np.asarray(b, np.float32)
    A = np.zeros((NN, NN), np.float32)
    np.add.at(A, (np.asarray(dst).astype(np.int64),
                  np.asarray(src).astype(np.int64)), 1.0)
    smalls = {
        "wt": np.ascontiguousarray(W.T),
        "gmat": np.ascontiguousarray(W.T @ W),
        "wsum": np.ascontiguousarray(W.sum(axis=0)[:, None]),
        "bwv": np.ascontiguousarray((W * b[:, None]).sum(axis=0)[:, None]),
        "bvec": np.ascontiguousarray(b[None, :]),
        "afl": np.ascontiguousarray(A.reshape(1, 9)),
        "gam": np.ascontiguousarray(np.asarray(gamma, np.float32)[None, :]),
        "bet": np.ascontiguousarray(np.asarray(beta, np.float32)[None, :]),
        "cst": np.array([[b_total * float(b.sum()),
                          b_total * float((b * b).sum()),
                          1.0 / (b_total * F),
                          BN_EPS]], np.float32),
    }
    return smalls


def _get_runner():
    global _runner
    with _runner_lock:
        if _runner is None:
            _runner = _Runner()
        return _runner


def kernel(h, W, b, gamma, beta, src, dst):
    h = np.asarray(h, np.float32)
    assert h.shape == (B_TOTAL, NN, F), h.shape
    runner = _get_runner()
    smalls = _host_prep(h, W, b, gamma, beta, src, dst, B_TOTAL)
    hf = np.ascontiguousarray(h.reshape(B_TOTAL, FW))
    in_maps = []
    for c in range(N_CORES):
        m = dict(smalls)
        m["h0"] = hf[c * B_LOC:(c + 1) * B_LOC]
        in_maps.append(m)
    outs = runner.run(in_maps)
    full = np.concatenate([outs[c]["out0"] for c in range(N_CORES)], axis=0)
    return full.reshape(B_TOTAL, NN, F)
